# revision 20
# baseline (speedup 1.0000x reference)
"""Q8 linear layer (dequant matmul) on 8 Trainium2 NeuronCores.

out[t, o] = sum_i (x[t, i] * scales[i]) * weight[o, i]

Sharding: tensor-parallel over out_features (14336 = 8 * 1792). Each core
gets the full pre-scaled activations and a 1792-column slice of weight^T.

Strategy (memory-bound; weight HBM traffic is the floor — ~380 B/ns/core
measured, so the 7.34 MB int8 stream is ~19.5 us):
  - Ship weights as 1 B/elem in a host-prearranged layout contiguous per
    SBUF partition. Uniform per k-tile: cols [0, 1344) packed two-per-
    uint16 (biased to unsigned, pair (m, 672+m)); cols [1344, 1792) plain
    int8. DVE unpacks the packed region ((w & 0xFF) | 0x6400 and
    (w >> 8) | 0x6400 — fp16 "magic" w + 1152 bit patterns); ACT converts
    the plain region (activation-Copy int8->fp16). The two engines run
    concurrently and their combined rate (~470 B/ns) beats the DMA rate.
  - Weight stream split into DMA groups [4x7, 2, 1] k-tiles plus two
    sub-k-tile groups for k31 on the SP HWDGE ring; the shrinking tail
    groups minimize the conversion lag after the last byte lands and
    partially hide the ~0.9us DMA completion-semaphore latency.
  - fp16 matmul (x pre-scaled to fp16 on host), packed4 col-groups
    (tile_position), accumulating into 4 PSUM banks [128, 448] over all
    32 k-tiles. Bank ob = output cols [448*ob, 448*(ob+1)); partition
    strip j holds k-tiles with k % 4 == j.
  - Final round runs ob-major; each PSUM bank is evacuated as fp16 the
    moment it closes: ob0/ob2 on DVE (tensor_scalar +bias), ob1 on ACT
    (activation-Identity +bias), ob3 on ACT (plain copy — cols >= 1344
    are never packed so carry no magic bias). Output leaves in two
    partition-major DMAs on separate HWDGE rings (banks 0-1 on SP,
    banks 2-3 on Act) so they process in parallel; host folds the 4
    partition strips.
  - The per-partition bias vector rides in the last 4 bytes of the xs
    tensor (single DMA).
"""

import os
import sys

for _p in ("/opt/trn_rl_repo", "/root/.axon_site/_ro/trn_rl_repo"):
    if os.path.isdir(_p) and _p not in sys.path:
        sys.path.insert(0, _p)

import numpy as np

import concourse.bass as bass
import concourse.mybir as mybir
import concourse.tile as tile
from concourse import bacc
from concourse.bass_utils import run_bass_kernel_spmd

TOKENS = 32
IN_F = 4096
OUT_F = 14336
NCORES = 8
OPC = OUT_F // NCORES  # 1792 out features per core
P = 128
KT = IN_F // P  # 32 k-tiles
OB = 4
OBS = OPC // OB  # 448 (one PSUM bank)

PACK_COLS = 1344  # columns converted via DVE uint16 bit-trick (w + 1152)
PACK_W = PACK_COLS // 2  # 672 uint16 words per k-tile
PLAIN_COLS = OPC - PACK_COLS  # 448 columns converted via ACT int8 copy
KB = OPC  # bytes per k-tile per partition (1344 packed + 448 plain)

# k-tiles per DMA group (k0-k30). DMA descriptors are per-partition and
# cost ~150ns of engine-slot each regardless of size, so small groups are
# overhead-bound (~1.2us each): keep the tail to ONE small group — k31 —
# which ships as a single extra DMA with bank-aligned packing so each
# bank's matmul waits only on its own converter.
GROUPS = [4, 4, 4, 4, 4, 4, 4, 3]
assert sum(GROUPS) == KT - 1
KL = KT - 1  # 31: the split k-tile
KLA = 896  # k31 sub-DMA A: cols 0:896 packed as pairs (m, 448+m) -> banks 0-1
KLB_PACK = 448  # k31 sub-DMA B: cols 896:1344 packed (pairs) -> bank 2
KLB_PLAIN = 448  # k31 sub-DMA B: cols 1344:1792 plain int8 -> bank 3

XS_COLS = KT * TOKENS  # 1024 fp16 cols of pre-scaled activations
XSB_COLS = XS_COLS + 2  # + 2 fp16 slots holding the fp32 bias

_cached_nc = {}


def _build():
    key = 0
    if key in _cached_nc:
        return _cached_nc[key]

    nc = bacc.Bacc(
        "TRN2", target_bir_lowering=False, debug=False, num_devices=NCORES
    )
    # pre-scaled activations [P, KT, TOKENS] + negated per-partition bias
    # (1152 * strip rowsums of xs, fp32) in the last 2 fp16 slots
    xsb = nc.dram_tensor("xsb", [P, XSB_COLS], mybir.dt.float16, kind="ExternalInput")
    w8 = nc.dram_tensor("w8", [P, KT * KB], mybir.dt.uint8, kind="ExternalInput")
    # partition-major output: one descriptor per partition
    outp = nc.dram_tensor(
        "outp", [P, OB * OBS], mybir.dt.float16, kind="ExternalOutput"
    )

    gstart = [sum(GROUPS[:i]) for i in range(len(GROUPS))]  # first k-tile of group
    w8_flat = w8.ap()

    with tile.TileContext(nc) as tc:
        with (
            tc.tile_pool(name="xpool", bufs=1) as xpool,
            tc.tile_pool(name="w8pool", bufs=len(GROUPS) + 2) as w8pool,
            tc.tile_pool(name="w16pool", bufs=8) as w16pool,
            tc.tile_pool(name="opool", bufs=1) as opool,
            tc.tile_pool(name="pspool", bufs=1, space=bass.MemorySpace.PSUM) as pspool,
        ):
            xs_sb = xpool.tile([P, XSB_COLS], mybir.dt.float16, name="xs_sb")
            w8_tiles = []
            for g, kg in enumerate(GROUPS):
                t8 = w8pool.tile([P, kg, KB], mybir.dt.uint8, name=f"w8_{g}", tag="w8")
                src = w8_flat[:, gstart[g] * KB : (gstart[g] + kg) * KB]
                nc.sync.dma_start(out=t8[:], in_=src)
                w8_tiles.append(t8)
                if g == 0:
                    # xs+bias queued right behind the first weight group: the
                    # weight stream starts immediately, xs lands before round 0
                    nc.sync.dma_start(out=xs_sb[:], in_=xsb.ap())
            # k31 arrives last as one DMA (1792B descriptors)
            t8l = w8pool.tile([P, KB], mybir.dt.uint8, name="w8_l", tag="w8")
            nc.sync.dma_start(out=t8l[:], in_=w8_flat[:, KL * KB : KT * KB])
            bias_ap = xs_sb[:, XS_COLS : XS_COLS + 2].bitcast(mybir.dt.float32)

            psums = [
                pspool.tile([P, OBS], mybir.dt.float32, name=f"ps_{ob}", tag=f"ps{ob}")
                for ob in range(OB)
            ]

            def unpack(out_ap, in_ap, hi):
                nc.vector.tensor_scalar(
                    out_ap,
                    in_ap,
                    8 if hi else 0x00FF,
                    0x6400,
                    mybir.AluOpType.logical_shift_right
                    if hi
                    else mybir.AluOpType.bitwise_and,
                    mybir.AluOpType.bitwise_or,
                )

            w16_tiles = []
            for g, kg in enumerate(GROUPS):
                t8 = w8_tiles[g]
                t16 = w16pool.tile(
                    [P, kg, OPC], mybir.dt.uint16, name=f"w16_{g}", tag="w16"
                )
                # packed cols 0:1344 on DVE (whole group per op — amortizes
                # the ~130ns/op fixed cost), plain cols 1344:1792 int8 on
                # ACT (whole group) — engines run parallel
                pk = t8[:, :, 0:PACK_COLS].bitcast(mybir.dt.uint16)
                unpack(t16[:, :, 0:PACK_W], pk, False)
                unpack(t16[:, :, PACK_W:PACK_COLS], pk, True)
                nc.scalar.copy(
                    t16[:, :, PACK_COLS:OPC].bitcast(mybir.dt.float16),
                    t8[:, :, PACK_COLS:OPC].bitcast(mybir.dt.int8),
                )
                w16_tiles.append(t16)

            # k31: three separate dst tiles so each bank's matmul waits only
            # on its own converter (DVE for banks 0-2, ACT for bank 3) and
            # the j=3 matmuls pipeline with the conversion ops
            t16a = w16pool.tile([P, KLA], mybir.dt.uint16, name="w16_la", tag="w16")
            pka = t8l[:, 0:KLA].bitcast(mybir.dt.uint16)
            unpack(t16a[:, 0 : KLA // 2], pka, False)
            unpack(t16a[:, KLA // 2 : KLA], pka, True)
            t16b1 = w16pool.tile(
                [P, KLB_PACK], mybir.dt.uint16, name="w16_lb1", tag="w16"
            )
            pkb = t8l[:, KLA : KLA + KLB_PACK].bitcast(mybir.dt.uint16)
            unpack(t16b1[:, 0 : KLB_PACK // 2], pkb, False)
            unpack(t16b1[:, KLB_PACK // 2 : KLB_PACK], pkb, True)
            t16b2 = w16pool.tile(
                [P, KLB_PLAIN], mybir.dt.float16, name="w16_lb2", tag="w16"
            )
            nc.scalar.copy(
                t16b2[:], t8l[:, PACK_COLS:OPC].bitcast(mybir.dt.int8)
            )

            # matmul rounds: round r consumes k-tiles 4r+j in col-group j
            def w16_ap(ki, ob):
                if ki == KL:
                    if ob < 2:
                        return t16a[:, ob * OBS : (ob + 1) * OBS].bitcast(
                            mybir.dt.float16
                        )
                    if ob == 2:
                        return t16b1[:, 0:OBS].bitcast(mybir.dt.float16)
                    return t16b2[:, 0:OBS]
                for g, kg in enumerate(GROUPS):
                    if gstart[g] <= ki < gstart[g] + kg:
                        return w16_tiles[g][
                            :, ki - gstart[g], ob * OBS : (ob + 1) * OBS
                        ].bitcast(mybir.dt.float16)
                raise AssertionError(ki)

            out_sb = opool.tile([P, OB, OBS], mybir.dt.float16, name="out_sb")

            def evac(ob):
                # fp16 partials; banks 0-2 hold packed cols -> subtract the
                # magic bias; bank 3 (cols 1344:1792) is never packed.
                # DVE takes banks 0/2, ACT banks 1/3 — two parallel waves.
                if ob == 3:
                    nc.scalar.copy(out_sb[:, 3, :], psums[3][:, :])
                elif ob == 1:
                    nc.scalar.activation(
                        out_sb[:, 1, :],
                        psums[1][:, :],
                        mybir.ActivationFunctionType.Identity,
                        bias=bias_ap,
                    )
                else:
                    nc.vector.tensor_scalar(
                        out_sb[:, ob, :],
                        psums[ob][:, :],
                        bias_ap,
                        None,
                        mybir.AluOpType.add,
                    )

            def mm(r, j, ob):
                nc.tensor.matmul(
                    psums[ob][32 * j : 32 * (j + 1), :],
                    xs_sb[:, (4 * r + j) * TOKENS : (4 * r + j + 1) * TOKENS],
                    w16_ap(4 * r + j, ob),
                    start=(r == 0),
                    stop=(r == nrounds - 1),
                    tile_position=(0, 32 * j),
                    # sim's zero-region group check drops the partition
                    # base of col-group strips; disjoint strips are safe
                    skip_group_check=True,
                )

            nrounds = KT // 4
            for r in range(nrounds - 1):
                for j in range(4):
                    for ob in range(OB):
                        mm(r, j, ob)
            # final round: j 0-2 consume k-tiles 28-30 (on-chip before k31)
            # and run while k31 converts; only the four j=3 matmuls sit
            # behind the final DMA. Each PSUM bank is evacuated the moment
            # it closes; output leaves via two parallel HWDGE rings.
            for j in range(3):
                for ob in range(OB):
                    mm(nrounds - 1, j, ob)
            for ob in range(OB):
                mm(nrounds - 1, 3, ob)
                evac(ob)
                if ob == 1:
                    nc.sync.dma_start(
                        out=outp.ap()[:, 0 : 2 * OBS], in_=out_sb[:, 0:2, :]
                    )
            # bank 3 rides the SP ring right behind banks 0-1; bank 2 rides
            # the Act ring — 229KB per ring, every trigger fires as soon as
            # its bank is evacuated. (Bank 2's trigger is emitted after
            # evac(3) so it doesn't block evac(3) on the Scalar engine.)
            nc.sync.dma_start(
                out=outp.ap()[:, 3 * OBS : 4 * OBS], in_=out_sb[:, 3, :]
            )
            nc.scalar.dma_start(
                out=outp.ap()[:, 2 * OBS : 3 * OBS], in_=out_sb[:, 2, :]
            )

    nc.compile()
    _cached_nc[key] = nc
    return nc


def make_in_maps(x, weight, scales):
    x = np.asarray(x, dtype=np.float32)
    weight = np.asarray(weight)
    scales = np.asarray(scales, dtype=np.float32)
    assert x.shape == (TOKENS, IN_F) and weight.shape == (OUT_F, IN_F)

    xs = x * scales[None, :]
    # [P, KT, TOKENS]: xsT[p, nk, t] = xs[t, nk*128 + p]
    xsT = np.ascontiguousarray(
        xs.T.reshape(KT, P, TOKENS).transpose(1, 0, 2)
    ).astype(np.float16)

    # negated magic-bias per psum partition 32j+t: strip j accumulates
    # k-tiles {4r+j}; packed cells contribute 1152 * xs per element
    xs16 = xsT.astype(np.float32)  # [P, KT, T]
    ksum = xs16.sum(axis=0).T  # [T, KT] per-k-tile rowsums
    bA = np.zeros((4, TOKENS), dtype=np.float32)
    for j in range(4):
        bA[j] = 1152.0 * ksum[:, [4 * r + j for r in range(KT // 4)]].sum(axis=1)
    bias = (-bA.reshape(P, 1)).astype(np.float32)  # [128, 1]

    xsb = np.concatenate(
        [xsT.reshape(P, XS_COLS), bias.view(np.float16)], axis=1
    )  # [P, XSB_COLS] fp16

    u8_full = (weight.astype(np.int16) + 128).astype(np.uint8)  # biased weights
    i8_full = weight.astype(np.int8)
    in_maps = []
    for c in range(NCORES):
        su = u8_full[c * OPC : (c + 1) * OPC, :]  # [OPC, IN_F] biased
        si = i8_full[c * OPC : (c + 1) * OPC, :]
        sut = su.T.reshape(KT, P, OPC)  # [KT, P, n]
        sit = si.T.reshape(KT, P, OPC)

        def pack_pairs(a):  # a: [..., 2*W] biased -> packed bytes [..., 2*W]
            w = a.shape[-1] // 2
            lo = a[..., 0:w].astype(np.uint16)
            hi = a[..., w : 2 * w].astype(np.uint16)
            return np.ascontiguousarray(lo | (hi << 8)).view(np.uint8)

        # k0-k30: 1344 packed bytes (pairs (m, 672+m)) + 448 plain int8.
        # k31: bank-aligned units — cols 0:896 packed (m, 448+m), cols
        # 896:1344 packed (m, 224+m), cols 1344:1792 plain int8.
        blob = np.concatenate(
            [
                pack_pairs(sut[:KL, :, 0:PACK_COLS]),
                sit[:KL, :, PACK_COLS:OPC].view(np.uint8),
            ],
            axis=2,
        )  # [KL, P, KB]
        k31 = np.concatenate(
            [
                pack_pairs(sut[KL, :, 0:KLA]),
                pack_pairs(sut[KL, :, KLA:PACK_COLS]),
                sit[KL, :, PACK_COLS:OPC].view(np.uint8),
            ],
            axis=1,
        )[None]  # [1, P, KB]
        blob = np.concatenate([blob, k31], axis=0)  # [KT, P, KB]
        w8c = np.ascontiguousarray(blob.transpose(1, 0, 2)).reshape(P, KT * KB)
        in_maps.append({"xsb": xsb, "w8": w8c})
    return in_maps


def run(x, weight, scales, trace=False, trace_cores=None, tmpdir=None):
    nc = _build()
    in_maps = make_in_maps(x, weight, scales)
    res = run_bass_kernel_spmd(
        nc,
        in_maps,
        core_ids=list(range(NCORES)),
        trace=trace,
        trace_cores=trace_cores,
        tmpdir=tmpdir,
    )
    cols = []
    for c in range(NCORES):
        part = (
            res.results[c]["outp"]
            .astype(np.float32)
            .reshape(4, TOKENS, OB, OBS)  # partition 32j+t -> (j, t)
        )
        folded = part.sum(axis=0)  # [TOKENS, OB, OBS]
        cols.append(folded.reshape(TOKENS, OPC))
    out = np.concatenate(cols, axis=1).astype(np.float32, copy=False)
    return out, res


def kernel(x, weight, scales):
    out, _ = run(x, weight, scales)
    return out


# revision 24
# speedup vs baseline: 1.0434x; 1.0434x over previous
"""Q8 linear layer (dequant matmul) on 8 Trainium2 NeuronCores.

out[t, o] = sum_i (x[t, i] * scales[i]) * weight[o, i]

Sharding: tensor-parallel over out_features (14336 = 8 * 1792). Each core
gets the full pre-scaled activations and a 1792-column slice of weight^T.

Strategy (memory-bound; weight HBM traffic is the floor — ~380 B/ns/core
measured, so the 7.34 MB int8 stream is ~19.5 us):
  - Ship weights as 1 B/elem in a host-prearranged layout contiguous per
    SBUF partition. Uniform per k-tile: cols [0, 1344) packed two-per-
    uint16 (biased to unsigned, pair (m, 672+m)); cols [1344, 1792) plain
    int8. DVE unpacks the packed region ((w & 0xFF) | 0x6400 and
    (w >> 8) | 0x6400 — fp16 "magic" w + 1152 bit patterns); ACT converts
    the plain region (activation-Copy int8->fp16). The two engines run
    concurrently and their combined rate (~470 B/ns) beats the DMA rate.
  - Weight stream split into DMA groups [4x7, 2, 1] k-tiles plus two
    sub-k-tile groups for k31 on the SP HWDGE ring; the shrinking tail
    groups minimize the conversion lag after the last byte lands and
    partially hide the ~0.9us DMA completion-semaphore latency.
  - fp16 matmul (x pre-scaled to fp16 on host), packed4 col-groups
    (tile_position), accumulating into 4 PSUM banks [128, 448] over all
    32 k-tiles. Bank ob = output cols [448*ob, 448*(ob+1)); partition
    strip j holds k-tiles with k % 4 == j.
  - Final round runs ob-major; each PSUM bank is evacuated as fp16 the
    moment it closes: ob0/ob2 on DVE (tensor_scalar +bias), ob1 on ACT
    (activation-Identity +bias), ob3 on ACT (plain copy — cols >= 1344
    are never packed so carry no magic bias). Output leaves in two
    partition-major DMAs on separate HWDGE rings (banks 0-1 on SP,
    banks 2-3 on Act) so they process in parallel; host folds the 4
    partition strips.
  - The per-partition bias vector rides in the last 4 bytes of the xs
    tensor (single DMA).
"""

import os
import sys

for _p in ("/opt/trn_rl_repo", "/root/.axon_site/_ro/trn_rl_repo"):
    if os.path.isdir(_p) and _p not in sys.path:
        sys.path.insert(0, _p)

import numpy as np

import concourse.bass as bass
import concourse.mybir as mybir
import concourse.tile as tile
from concourse import bacc
from concourse.bass_utils import run_bass_kernel_spmd

TOKENS = 32
IN_F = 4096
OUT_F = 14336
NCORES = 8
OPC = OUT_F // NCORES  # 1792 out features per core
P = 128
KT = IN_F // P  # 32 k-tiles
OB = 4
OBS = OPC // OB  # 448 (one PSUM bank)

PACK_COLS = 1344  # columns converted via DVE uint16 bit-trick (w + 1152)
PACK_W = PACK_COLS // 2  # 672 uint16 words per k-tile
PLAIN_COLS = OPC - PACK_COLS  # 448 columns converted via ACT int8 copy
KB = OPC  # bytes per k-tile per partition (1344 packed + 448 plain)

# k-tiles per DMA group (k0-k29). DMA descriptors are per-partition and
# cost ~150ns of engine-slot each regardless of size, so 1-k-tile groups
# are overhead-bound (~1.2us): the tail ships k30+k31 together as one
# 2-k-tile DMA (3584B descriptors, bandwidth-efficient); k31 uses
# bank-aligned packing so each bank's matmul waits only on its own
# converter and pipelines with the conversion ops.
GROUPS = [4, 4, 4, 4, 4, 4, 4, 2]
assert sum(GROUPS) == KT - 2
KP = KT - 2  # 30: second-to-last k-tile, rides in the last DMA
KL = KT - 1  # 31: the split k-tile
KLA = 896  # k31 sub-DMA A: cols 0:896 packed as pairs (m, 448+m) -> banks 0-1
KLB_PACK = 448  # k31 sub-DMA B: cols 896:1344 packed (pairs) -> bank 2
KLB_PLAIN = 448  # k31 sub-DMA B: cols 1344:1792 plain int8 -> bank 3

XS_COLS = KT * TOKENS  # 1024 fp16 cols of pre-scaled activations
XSB_COLS = XS_COLS + 2  # + 2 fp16 slots holding the fp32 bias

_cached_nc = {}


def _build():
    key = 0
    if key in _cached_nc:
        return _cached_nc[key]

    nc = bacc.Bacc(
        "TRN2", target_bir_lowering=False, debug=False, num_devices=NCORES
    )
    # pre-scaled activations [P, KT, TOKENS] + negated per-partition bias
    # (1152 * strip rowsums of xs, fp32) in the last 2 fp16 slots
    xsb = nc.dram_tensor("xsb", [P, XSB_COLS], mybir.dt.float16, kind="ExternalInput")
    w8 = nc.dram_tensor("w8", [P, KT * KB], mybir.dt.uint8, kind="ExternalInput")
    # partition-major output: one descriptor per partition
    outp = nc.dram_tensor(
        "outp", [P, OB * OBS], mybir.dt.float16, kind="ExternalOutput"
    )

    gstart = [sum(GROUPS[:i]) for i in range(len(GROUPS))]  # first k-tile of group
    w8_flat = w8.ap()

    with tile.TileContext(nc) as tc:
        with (
            tc.tile_pool(name="xpool", bufs=1) as xpool,
            tc.tile_pool(name="w8pool", bufs=len(GROUPS) + 2) as w8pool,
            tc.tile_pool(name="w16pool", bufs=8) as w16pool,
            tc.tile_pool(name="opool", bufs=1) as opool,
            tc.tile_pool(name="pspool", bufs=1, space=bass.MemorySpace.PSUM) as pspool,
        ):
            xs_sb = xpool.tile([P, XSB_COLS], mybir.dt.float16, name="xs_sb")
            w8_tiles = []
            for g, kg in enumerate(GROUPS):
                t8 = w8pool.tile([P, kg, KB], mybir.dt.uint8, name=f"w8_{g}", tag="w8")
                src = w8_flat[:, gstart[g] * KB : (gstart[g] + kg) * KB]
                nc.sync.dma_start(out=t8[:], in_=src)
                w8_tiles.append(t8)
                if g == 0:
                    # xs+bias queued right behind the first weight group: the
                    # weight stream starts immediately, xs lands before round 0
                    nc.sync.dma_start(out=xs_sb[:], in_=xsb.ap())
            # k30+k31 arrive last as one DMA (3584B descriptors)
            t8l = w8pool.tile([P, 2, KB], mybir.dt.uint8, name="w8_l", tag="w8")
            nc.sync.dma_start(out=t8l[:], in_=w8_flat[:, KP * KB : KT * KB])
            bias_ap = xs_sb[:, XS_COLS : XS_COLS + 2].bitcast(mybir.dt.float32)

            psums = [
                pspool.tile([P, OBS], mybir.dt.float32, name=f"ps_{ob}", tag=f"ps{ob}")
                for ob in range(OB)
            ]

            def unpack(out_ap, in_ap, hi):
                nc.vector.tensor_scalar(
                    out_ap,
                    in_ap,
                    8 if hi else 0x00FF,
                    0x6400,
                    mybir.AluOpType.logical_shift_right
                    if hi
                    else mybir.AluOpType.bitwise_and,
                    mybir.AluOpType.bitwise_or,
                )

            w16_tiles = []
            for g, kg in enumerate(GROUPS):
                t8 = w8_tiles[g]
                t16 = w16pool.tile(
                    [P, kg, OPC], mybir.dt.uint16, name=f"w16_{g}", tag="w16"
                )
                # packed cols 0:1344 on DVE (whole group per op — amortizes
                # the ~130ns/op fixed cost), plain cols 1344:1792 int8 on
                # ACT (whole group) — engines run parallel
                pk = t8[:, :, 0:PACK_COLS].bitcast(mybir.dt.uint16)
                unpack(t16[:, :, 0:PACK_W], pk, False)
                unpack(t16[:, :, PACK_W:PACK_COLS], pk, True)
                nc.scalar.copy(
                    t16[:, :, PACK_COLS:OPC].bitcast(mybir.dt.float16),
                    t8[:, :, PACK_COLS:OPC].bitcast(mybir.dt.int8),
                )
                w16_tiles.append(t16)

            # k30: standard layout, converted first (its j=2 matmuls precede
            # k31's in the final round)
            t16p = w16pool.tile([P, OPC], mybir.dt.uint16, name="w16_p", tag="w16")
            pkp = t8l[:, 0, 0:PACK_COLS].bitcast(mybir.dt.uint16)
            unpack(t16p[:, 0:PACK_W], pkp, False)
            unpack(t16p[:, PACK_W:PACK_COLS], pkp, True)
            nc.scalar.copy(
                t16p[:, PACK_COLS:OPC].bitcast(mybir.dt.float16),
                t8l[:, 0, PACK_COLS:OPC].bitcast(mybir.dt.int8),
            )
            # k31: three separate dst tiles so each bank's matmul waits only
            # on its own converter (DVE for banks 0-2, ACT for bank 3) and
            # the j=3 matmuls pipeline with the conversion ops
            t16a = w16pool.tile([P, KLA], mybir.dt.uint16, name="w16_la", tag="w16")
            pka = t8l[:, 1, 0:KLA].bitcast(mybir.dt.uint16)
            unpack(t16a[:, 0 : KLA // 2], pka, False)
            unpack(t16a[:, KLA // 2 : KLA], pka, True)
            t16b1 = w16pool.tile(
                [P, KLB_PACK], mybir.dt.uint16, name="w16_lb1", tag="w16"
            )
            pkb = t8l[:, 1, KLA : KLA + KLB_PACK].bitcast(mybir.dt.uint16)
            unpack(t16b1[:, 0 : KLB_PACK // 2], pkb, False)
            unpack(t16b1[:, KLB_PACK // 2 : KLB_PACK], pkb, True)
            t16b2 = w16pool.tile(
                [P, KLB_PLAIN], mybir.dt.float16, name="w16_lb2", tag="w16"
            )
            nc.scalar.copy(
                t16b2[:], t8l[:, 1, PACK_COLS:OPC].bitcast(mybir.dt.int8)
            )

            # matmul rounds: round r consumes k-tiles 4r+j in col-group j
            def w16_ap(ki, ob):
                if ki == KP:
                    return t16p[:, ob * OBS : (ob + 1) * OBS].bitcast(
                        mybir.dt.float16
                    )
                if ki == KL:
                    if ob < 2:
                        return t16a[:, ob * OBS : (ob + 1) * OBS].bitcast(
                            mybir.dt.float16
                        )
                    if ob == 2:
                        return t16b1[:, 0:OBS].bitcast(mybir.dt.float16)
                    return t16b2[:, 0:OBS]
                for g, kg in enumerate(GROUPS):
                    if gstart[g] <= ki < gstart[g] + kg:
                        return w16_tiles[g][
                            :, ki - gstart[g], ob * OBS : (ob + 1) * OBS
                        ].bitcast(mybir.dt.float16)
                raise AssertionError(ki)

            out_sb = opool.tile([P, OB, OBS], mybir.dt.float16, name="out_sb")

            def evac(ob):
                # fp16 partials; banks 0-2 hold packed cols -> subtract the
                # magic bias; bank 3 (cols 1344:1792) is never packed.
                # DVE takes banks 0/2, ACT banks 1/3 — two parallel waves.
                if ob == 3:
                    nc.scalar.copy(out_sb[:, 3, :], psums[3][:, :])
                elif ob == 1:
                    nc.scalar.activation(
                        out_sb[:, 1, :],
                        psums[1][:, :],
                        mybir.ActivationFunctionType.Identity,
                        bias=bias_ap,
                    )
                else:
                    nc.vector.tensor_scalar(
                        out_sb[:, ob, :],
                        psums[ob][:, :],
                        bias_ap,
                        None,
                        mybir.AluOpType.add,
                    )

            def mm(r, j, ob):
                nc.tensor.matmul(
                    psums[ob][32 * j : 32 * (j + 1), :],
                    xs_sb[:, (4 * r + j) * TOKENS : (4 * r + j + 1) * TOKENS],
                    w16_ap(4 * r + j, ob),
                    start=(r == 0),
                    stop=(r == nrounds - 1),
                    tile_position=(0, 32 * j),
                    # sim's zero-region group check drops the partition
                    # base of col-group strips; disjoint strips are safe
                    skip_group_check=True,
                )

            nrounds = KT // 4
            for r in range(nrounds - 1):
                for j in range(4):
                    for ob in range(OB):
                        mm(r, j, ob)
            # final round: j 0-2 consume k-tiles 28-30 (on-chip before k31)
            # and run while k31 converts; only the four j=3 matmuls sit
            # behind the final DMA. Each PSUM bank is evacuated the moment
            # it closes; output leaves via two parallel HWDGE rings.
            for j in range(3):
                for ob in range(OB):
                    mm(nrounds - 1, j, ob)
            for ob in range(OB):
                mm(nrounds - 1, 3, ob)
                evac(ob)
                if ob == 1:
                    nc.sync.dma_start(
                        out=outp.ap()[:, 0 : 2 * OBS], in_=out_sb[:, 0:2, :]
                    )
            # bank 3 rides the SP ring right behind banks 0-1; bank 2 rides
            # the Act ring — 229KB per ring, every trigger fires as soon as
            # its bank is evacuated. (Bank 2's trigger is emitted after
            # evac(3) so it doesn't block evac(3) on the Scalar engine.)
            nc.sync.dma_start(
                out=outp.ap()[:, 3 * OBS : 4 * OBS], in_=out_sb[:, 3, :]
            )
            nc.scalar.dma_start(
                out=outp.ap()[:, 2 * OBS : 3 * OBS], in_=out_sb[:, 2, :]
            )

    nc.compile()
    _cached_nc[key] = nc
    return nc


def make_in_maps(x, weight, scales):
    x = np.asarray(x, dtype=np.float32)
    weight = np.asarray(weight)
    scales = np.asarray(scales, dtype=np.float32)
    assert x.shape == (TOKENS, IN_F) and weight.shape == (OUT_F, IN_F)

    xs = x * scales[None, :]
    # [P, KT, TOKENS]: xsT[p, nk, t] = xs[t, nk*128 + p]
    xsT = np.ascontiguousarray(
        xs.T.reshape(KT, P, TOKENS).transpose(1, 0, 2)
    ).astype(np.float16)

    # negated magic-bias per psum partition 32j+t: strip j accumulates
    # k-tiles {4r+j}; packed cells contribute 1152 * xs per element
    xs16 = xsT.astype(np.float32)  # [P, KT, T]
    ksum = xs16.sum(axis=0).T  # [T, KT] per-k-tile rowsums
    bA = np.zeros((4, TOKENS), dtype=np.float32)
    for j in range(4):
        bA[j] = 1152.0 * ksum[:, [4 * r + j for r in range(KT // 4)]].sum(axis=1)
    bias = (-bA.reshape(P, 1)).astype(np.float32)  # [128, 1]

    xsb = np.concatenate(
        [xsT.reshape(P, XS_COLS), bias.view(np.float16)], axis=1
    )  # [P, XSB_COLS] fp16

    u8_full = (weight.astype(np.int16) + 128).astype(np.uint8)  # biased weights
    i8_full = weight.astype(np.int8)
    in_maps = []
    for c in range(NCORES):
        su = u8_full[c * OPC : (c + 1) * OPC, :]  # [OPC, IN_F] biased
        si = i8_full[c * OPC : (c + 1) * OPC, :]
        sut = su.T.reshape(KT, P, OPC)  # [KT, P, n]
        sit = si.T.reshape(KT, P, OPC)

        def pack_pairs(a):  # a: [..., 2*W] biased -> packed bytes [..., 2*W]
            w = a.shape[-1] // 2
            lo = a[..., 0:w].astype(np.uint16)
            hi = a[..., w : 2 * w].astype(np.uint16)
            return np.ascontiguousarray(lo | (hi << 8)).view(np.uint8)

        # k0-k30: 1344 packed bytes (pairs (m, 672+m)) + 448 plain int8.
        # k31: bank-aligned units — cols 0:896 packed (m, 448+m), cols
        # 896:1344 packed (m, 224+m), cols 1344:1792 plain int8.
        blob = np.concatenate(
            [
                pack_pairs(sut[:KL, :, 0:PACK_COLS]),
                sit[:KL, :, PACK_COLS:OPC].view(np.uint8),
            ],
            axis=2,
        )  # [KL, P, KB]
        k31 = np.concatenate(
            [
                pack_pairs(sut[KL, :, 0:KLA]),
                pack_pairs(sut[KL, :, KLA:PACK_COLS]),
                sit[KL, :, PACK_COLS:OPC].view(np.uint8),
            ],
            axis=1,
        )[None]  # [1, P, KB]
        blob = np.concatenate([blob, k31], axis=0)  # [KT, P, KB]
        w8c = np.ascontiguousarray(blob.transpose(1, 0, 2)).reshape(P, KT * KB)
        in_maps.append({"xsb": xsb, "w8": w8c})
    return in_maps


def run(x, weight, scales, trace=False, trace_cores=None, tmpdir=None):
    nc = _build()
    in_maps = make_in_maps(x, weight, scales)
    res = run_bass_kernel_spmd(
        nc,
        in_maps,
        core_ids=list(range(NCORES)),
        trace=trace,
        trace_cores=trace_cores,
        tmpdir=tmpdir,
    )
    cols = []
    for c in range(NCORES):
        part = (
            res.results[c]["outp"]
            .astype(np.float32)
            .reshape(4, TOKENS, OB, OBS)  # partition 32j+t -> (j, t)
        )
        folded = part.sum(axis=0)  # [TOKENS, OB, OBS]
        cols.append(folded.reshape(TOKENS, OPC))
    out = np.concatenate(cols, axis=1).astype(np.float32, copy=False)
    return out, res


def kernel(x, weight, scales):
    out, _ = run(x, weight, scales)
    return out


# revision 28
# speedup vs baseline: 1.1162x; 1.0697x over previous
"""Q8 linear layer (dequant matmul) on 8 Trainium2 NeuronCores.

out[t, o] = sum_i (x[t, i] * scales[i]) * weight[o, i]

Sharding: tensor-parallel over out_features (14336 = 8 * 1792). Each core
gets the full pre-scaled activations and a 1792-column slice of weight^T.

Strategy (memory-bound; weight HBM traffic is the floor — ~380 B/ns/core
measured, so the 7.34 MB int8 stream is ~19.5 us):
  - Ship weights as 1 B/elem in a host-prearranged layout contiguous per
    SBUF partition. Uniform per k-tile: cols [0, 1344) packed two-per-
    uint16 (biased to unsigned, pair (m, 672+m)); cols [1344, 1792) plain
    int8. DVE unpacks the packed region ((w & 0xFF) | 0x6400 and
    (w >> 8) | 0x6400 — fp16 "magic" w + 1152 bit patterns); ACT converts
    the plain region (activation-Copy int8->fp16). The two engines run
    concurrently and their combined rate (~470 B/ns) beats the DMA rate.
  - Weight stream split into DMA groups [4x7, 2, 1] k-tiles plus two
    sub-k-tile groups for k31 on the SP HWDGE ring; the shrinking tail
    groups minimize the conversion lag after the last byte lands and
    partially hide the ~0.9us DMA completion-semaphore latency.
  - fp16 matmul (x pre-scaled to fp16 on host), packed4 col-groups
    (tile_position), accumulating into 4 PSUM banks [128, 448] over all
    32 k-tiles. Bank ob = output cols [448*ob, 448*(ob+1)); partition
    strip j holds k-tiles with k % 4 == j.
  - Final round runs ob-major; each PSUM bank is evacuated as fp16 the
    moment it closes: ob0/ob2 on DVE (tensor_scalar +bias), ob1 on ACT
    (activation-Identity +bias), ob3 on ACT (plain copy — cols >= 1344
    are never packed so carry no magic bias). Output leaves in two
    partition-major DMAs on separate HWDGE rings (banks 0-1 on SP,
    banks 2-3 on Act) so they process in parallel; host folds the 4
    partition strips.
  - The per-partition bias vector rides in the last 4 bytes of the xs
    tensor (single DMA).
"""

import os
import sys

for _p in ("/opt/trn_rl_repo", "/root/.axon_site/_ro/trn_rl_repo"):
    if os.path.isdir(_p) and _p not in sys.path:
        sys.path.insert(0, _p)

import numpy as np

import concourse.bass as bass
import concourse.mybir as mybir
import concourse.tile as tile
from concourse import bacc
from concourse.bass_utils import run_bass_kernel_spmd

TOKENS = 32
IN_F = 4096
OUT_F = 14336
NCORES = 8
OPC = OUT_F // NCORES  # 1792 out features per core
P = 128
KT = IN_F // P  # 32 k-tiles
OB = 4
OBS = OPC // OB  # 448 (one PSUM bank)

PACK_COLS = 1344  # columns converted via DVE uint16 bit-trick (w + 1152)
PACK_W = PACK_COLS // 2  # 672 uint16 words per k-tile
PLAIN_COLS = OPC - PACK_COLS  # 448 columns converted via ACT int8 copy
KB = OPC  # bytes per k-tile per partition (1344 packed + 448 plain)

# k-tiles per DMA group (k0-k29). DMA descriptors are per-partition and
# cost ~150ns of engine-slot each regardless of size, so 1-k-tile groups
# are overhead-bound (~1.2us): the tail ships k30+k31 together as one
# 2-k-tile DMA (3584B descriptors, bandwidth-efficient); k31 uses
# bank-aligned packing so each bank's matmul waits only on its own
# converter and pipelines with the conversion ops.
GROUPS = [4, 4, 4, 4, 4, 4, 4, 2]
assert sum(GROUPS) == KT - 2
KP = KT - 2  # 30: second-to-last k-tile, rides in the last DMA
KL = KT - 1  # 31: the split k-tile
KLA = 896  # k31 sub-DMA A: cols 0:896 packed as pairs (m, 448+m) -> banks 0-1
KLB_PACK = 448  # k31 sub-DMA B: cols 896:1344 packed (pairs) -> bank 2
KLB_PLAIN = 448  # k31 sub-DMA B: cols 1344:1792 plain int8 -> bank 3

XS_COLS = KT * TOKENS  # 1024 fp16 cols of pre-scaled activations
XSB_COLS = XS_COLS + 2  # + 2 fp16 slots holding the fp32 bias
XSB_BYTES = 2 * XSB_COLS  # 2052 B, rides at the head of g0's DMA

_cached_nc = {}


def _build():
    key = 0
    if key in _cached_nc:
        return _cached_nc[key]

    nc = bacc.Bacc(
        "TRN2", target_bir_lowering=False, debug=False, num_devices=NCORES
    )
    # weights, prefixed per partition by the pre-scaled activations
    # [KT, TOKENS] fp16 + negated per-partition bias (1152 * strip rowsums
    # of xs, fp32) — the xs bytes ride inside g0's DMA descriptors so they
    # don't cost a separate overhead-bound transfer
    w8 = nc.dram_tensor(
        "w8", [P, XSB_BYTES + KT * KB], mybir.dt.uint8, kind="ExternalInput"
    )
    # partition-major output: one descriptor per partition
    outp = nc.dram_tensor(
        "outp", [P, OB * OBS], mybir.dt.float16, kind="ExternalOutput"
    )

    gstart = [sum(GROUPS[:i]) for i in range(len(GROUPS))]  # first k-tile of group
    w8_flat = w8.ap()

    with tile.TileContext(nc) as tc:
        with (
            tc.tile_pool(name="xpool", bufs=1) as xpool,
            tc.tile_pool(name="w8pool", bufs=len(GROUPS) + 2) as w8pool,
            tc.tile_pool(name="w16pool", bufs=8) as w16pool,
            tc.tile_pool(name="opool", bufs=1) as opool,
            tc.tile_pool(name="pspool", bufs=1, space=bass.MemorySpace.PSUM) as pspool,
        ):
            w8_tiles = []
            for g, kg in enumerate(GROUPS):
                if g == 0:
                    # g0 carries xs+bias (2052 B) + its 4 k-tiles in one DMA
                    t8g0 = w8pool.tile(
                        [P, XSB_BYTES + 4 * KB], mybir.dt.uint8, name="w8_0", tag="w8"
                    )
                    nc.sync.dma_start(
                        out=t8g0[:], in_=w8_flat[:, 0 : XSB_BYTES + 4 * KB]
                    )
                    w8_tiles.append(
                        t8g0[:, XSB_BYTES:].rearrange("p (k b) -> p k b", k=4)
                    )
                    continue
                t8 = w8pool.tile([P, kg, KB], mybir.dt.uint8, name=f"w8_{g}", tag="w8")
                src = w8_flat[
                    :, XSB_BYTES + gstart[g] * KB : XSB_BYTES + (gstart[g] + kg) * KB
                ]
                nc.sync.dma_start(out=t8[:], in_=src)
                w8_tiles.append(t8)
            # k30+k31 arrive last as one DMA (3584B descriptors)
            t8l = w8pool.tile([P, 2, KB], mybir.dt.uint8, name="w8_l", tag="w8")
            nc.sync.dma_start(
                out=t8l[:], in_=w8_flat[:, XSB_BYTES + KP * KB : XSB_BYTES + KT * KB]
            )
            xs_sb = t8g0[:, 0:XSB_BYTES].bitcast(mybir.dt.float16)
            bias_ap = t8g0[:, 2 * XS_COLS : XSB_BYTES].bitcast(mybir.dt.float32)

            psums = [
                pspool.tile([P, OBS], mybir.dt.float32, name=f"ps_{ob}", tag=f"ps{ob}")
                for ob in range(OB)
            ]

            def unpack(out_ap, in_ap, hi):
                nc.vector.tensor_scalar(
                    out_ap,
                    in_ap,
                    8 if hi else 0x00FF,
                    0x6400,
                    mybir.AluOpType.logical_shift_right
                    if hi
                    else mybir.AluOpType.bitwise_and,
                    mybir.AluOpType.bitwise_or,
                )

            w16_tiles = []
            for g, kg in enumerate(GROUPS):
                t8 = w8_tiles[g]
                t16 = w16pool.tile(
                    [P, kg, OPC], mybir.dt.uint16, name=f"w16_{g}", tag="w16"
                )
                # packed cols 0:1344 on DVE (whole group per op — amortizes
                # the ~130ns/op fixed cost), plain cols 1344:1792 int8 on
                # ACT (whole group) — engines run parallel
                pk = t8[:, :, 0:PACK_COLS].bitcast(mybir.dt.uint16)
                unpack(t16[:, :, 0:PACK_W], pk, False)
                unpack(t16[:, :, PACK_W:PACK_COLS], pk, True)
                nc.scalar.copy(
                    t16[:, :, PACK_COLS:OPC].bitcast(mybir.dt.float16),
                    t8[:, :, PACK_COLS:OPC].bitcast(mybir.dt.int8),
                )
                w16_tiles.append(t16)

            # k30: standard layout, converted first (its j=2 matmuls precede
            # k31's in the final round)
            t16p = w16pool.tile([P, OPC], mybir.dt.uint16, name="w16_p", tag="w16")
            pkp = t8l[:, 0, 0:PACK_COLS].bitcast(mybir.dt.uint16)
            unpack(t16p[:, 0:PACK_W], pkp, False)
            unpack(t16p[:, PACK_W:PACK_COLS], pkp, True)
            nc.scalar.copy(
                t16p[:, PACK_COLS:OPC].bitcast(mybir.dt.float16),
                t8l[:, 0, PACK_COLS:OPC].bitcast(mybir.dt.int8),
            )
            # k31: three separate dst tiles so each bank's matmul waits only
            # on its own converter (DVE for banks 0-2, ACT for bank 3) and
            # the j=3 matmuls pipeline with the conversion ops
            t16a = w16pool.tile([P, KLA], mybir.dt.uint16, name="w16_la", tag="w16")
            pka = t8l[:, 1, 0:KLA].bitcast(mybir.dt.uint16)
            unpack(t16a[:, 0 : KLA // 2], pka, False)
            unpack(t16a[:, KLA // 2 : KLA], pka, True)
            t16b1 = w16pool.tile(
                [P, KLB_PACK], mybir.dt.uint16, name="w16_lb1", tag="w16"
            )
            pkb = t8l[:, 1, KLA : KLA + KLB_PACK].bitcast(mybir.dt.uint16)
            unpack(t16b1[:, 0 : KLB_PACK // 2], pkb, False)
            unpack(t16b1[:, KLB_PACK // 2 : KLB_PACK], pkb, True)
            t16b2 = w16pool.tile(
                [P, KLB_PLAIN], mybir.dt.float16, name="w16_lb2", tag="w16"
            )
            nc.scalar.copy(
                t16b2[:], t8l[:, 1, PACK_COLS:OPC].bitcast(mybir.dt.int8)
            )

            # matmul rounds: round r consumes k-tiles 4r+j in col-group j
            def w16_ap(ki, ob):
                if ki == KP:
                    return t16p[:, ob * OBS : (ob + 1) * OBS].bitcast(
                        mybir.dt.float16
                    )
                if ki == KL:
                    if ob < 2:
                        return t16a[:, ob * OBS : (ob + 1) * OBS].bitcast(
                            mybir.dt.float16
                        )
                    if ob == 2:
                        return t16b1[:, 0:OBS].bitcast(mybir.dt.float16)
                    return t16b2[:, 0:OBS]
                for g, kg in enumerate(GROUPS):
                    if gstart[g] <= ki < gstart[g] + kg:
                        return w16_tiles[g][
                            :, ki - gstart[g], ob * OBS : (ob + 1) * OBS
                        ].bitcast(mybir.dt.float16)
                raise AssertionError(ki)

            out_sb = opool.tile([P, OB, OBS], mybir.dt.float16, name="out_sb")

            def evac(ob):
                # fp16 partials; banks 0-2 hold packed cols -> subtract the
                # magic bias; bank 3 (cols 1344:1792) is never packed.
                # DVE takes banks 0/2, ACT banks 1/3 — two parallel waves.
                if ob == 3:
                    nc.scalar.copy(out_sb[:, 3, :], psums[3][:, :])
                elif ob == 1:
                    nc.scalar.activation(
                        out_sb[:, 1, :],
                        psums[1][:, :],
                        mybir.ActivationFunctionType.Identity,
                        bias=bias_ap,
                    )
                else:
                    nc.vector.tensor_scalar(
                        out_sb[:, ob, :],
                        psums[ob][:, :],
                        bias_ap,
                        None,
                        mybir.AluOpType.add,
                    )

            def mm(r, j, ob):
                nc.tensor.matmul(
                    psums[ob][32 * j : 32 * (j + 1), :],
                    xs_sb[:, (4 * r + j) * TOKENS : (4 * r + j + 1) * TOKENS],
                    w16_ap(4 * r + j, ob),
                    start=(r == 0),
                    stop=(r == nrounds - 1),
                    tile_position=(0, 32 * j),
                    # sim's zero-region group check drops the partition
                    # base of col-group strips; disjoint strips are safe
                    skip_group_check=True,
                )

            nrounds = KT // 4
            for r in range(nrounds - 1):
                for j in range(4):
                    for ob in range(OB):
                        mm(r, j, ob)
            # final round: j 0-2 consume k-tiles 28-30 (on-chip before k31)
            # and run while k31 converts; only the four j=3 matmuls sit
            # behind the final DMA. Each PSUM bank is evacuated the moment
            # it closes; output leaves via two parallel HWDGE rings.
            for j in range(3):
                for ob in range(OB):
                    mm(nrounds - 1, j, ob)
            for ob in range(OB):
                mm(nrounds - 1, 3, ob)
                evac(ob)
                if ob == 1:
                    nc.sync.dma_start(
                        out=outp.ap()[:, 0 : 2 * OBS], in_=out_sb[:, 0:2, :]
                    )
            # bank 3 rides the SP ring right behind banks 0-1; bank 2 rides
            # the Act ring — 229KB per ring, every trigger fires as soon as
            # its bank is evacuated. (Bank 2's trigger is emitted after
            # evac(3) so it doesn't block evac(3) on the Scalar engine.)
            nc.sync.dma_start(
                out=outp.ap()[:, 3 * OBS : 4 * OBS], in_=out_sb[:, 3, :]
            )
            nc.scalar.dma_start(
                out=outp.ap()[:, 2 * OBS : 3 * OBS], in_=out_sb[:, 2, :]
            )

    nc.compile()
    _cached_nc[key] = nc
    return nc


def make_in_maps(x, weight, scales):
    x = np.asarray(x, dtype=np.float32)
    weight = np.asarray(weight)
    scales = np.asarray(scales, dtype=np.float32)
    assert x.shape == (TOKENS, IN_F) and weight.shape == (OUT_F, IN_F)

    xs = x * scales[None, :]
    # [P, KT, TOKENS]: xsT[p, nk, t] = xs[t, nk*128 + p]
    xsT = np.ascontiguousarray(
        xs.T.reshape(KT, P, TOKENS).transpose(1, 0, 2)
    ).astype(np.float16)

    # negated magic-bias per psum partition 32j+t: strip j accumulates
    # k-tiles {4r+j}; packed cells contribute 1152 * xs per element
    xs16 = xsT.astype(np.float32)  # [P, KT, T]
    ksum = xs16.sum(axis=0).T  # [T, KT] per-k-tile rowsums
    bA = np.zeros((4, TOKENS), dtype=np.float32)
    for j in range(4):
        bA[j] = 1152.0 * ksum[:, [4 * r + j for r in range(KT // 4)]].sum(axis=1)
    bias = (-bA.reshape(P, 1)).astype(np.float32)  # [128, 1]

    xsb = np.concatenate(
        [xsT.reshape(P, XS_COLS), bias.view(np.float16)], axis=1
    )  # [P, XSB_COLS] fp16

    u8_full = (weight.astype(np.int16) + 128).astype(np.uint8)  # biased weights
    i8_full = weight.astype(np.int8)
    in_maps = []
    for c in range(NCORES):
        su = u8_full[c * OPC : (c + 1) * OPC, :]  # [OPC, IN_F] biased
        si = i8_full[c * OPC : (c + 1) * OPC, :]
        sut = su.T.reshape(KT, P, OPC)  # [KT, P, n]
        sit = si.T.reshape(KT, P, OPC)

        def pack_pairs(a):  # a: [..., 2*W] biased -> packed bytes [..., 2*W]
            w = a.shape[-1] // 2
            lo = a[..., 0:w].astype(np.uint16)
            hi = a[..., w : 2 * w].astype(np.uint16)
            return np.ascontiguousarray(lo | (hi << 8)).view(np.uint8)

        # k0-k30: 1344 packed bytes (pairs (m, 672+m)) + 448 plain int8.
        # k31: bank-aligned units — cols 0:896 packed (m, 448+m), cols
        # 896:1344 packed (m, 224+m), cols 1344:1792 plain int8.
        blob = np.concatenate(
            [
                pack_pairs(sut[:KL, :, 0:PACK_COLS]),
                sit[:KL, :, PACK_COLS:OPC].view(np.uint8),
            ],
            axis=2,
        )  # [KL, P, KB]
        k31 = np.concatenate(
            [
                pack_pairs(sut[KL, :, 0:KLA]),
                pack_pairs(sut[KL, :, KLA:PACK_COLS]),
                sit[KL, :, PACK_COLS:OPC].view(np.uint8),
            ],
            axis=1,
        )[None]  # [1, P, KB]
        blob = np.concatenate([blob, k31], axis=0)  # [KT, P, KB]
        w8c = np.ascontiguousarray(blob.transpose(1, 0, 2)).reshape(P, KT * KB)
        # xs+bias bytes prefix each partition row (ride in g0's descriptors)
        w8c = np.concatenate([xsb.view(np.uint8), w8c], axis=1)
        in_maps.append({"w8": w8c})
    return in_maps


def run(x, weight, scales, trace=False, trace_cores=None, tmpdir=None):
    nc = _build()
    in_maps = make_in_maps(x, weight, scales)
    res = run_bass_kernel_spmd(
        nc,
        in_maps,
        core_ids=list(range(NCORES)),
        trace=trace,
        trace_cores=trace_cores,
        tmpdir=tmpdir,
    )
    cols = []
    for c in range(NCORES):
        part = (
            res.results[c]["outp"]
            .astype(np.float32)
            .reshape(4, TOKENS, OB, OBS)  # partition 32j+t -> (j, t)
        )
        folded = part.sum(axis=0)  # [TOKENS, OB, OBS]
        cols.append(folded.reshape(TOKENS, OPC))
    out = np.concatenate(cols, axis=1).astype(np.float32, copy=False)
    return out, res


def kernel(x, weight, scales):
    out, _ = run(x, weight, scales)
    return out
